# revision 8
# baseline (speedup 1.0000x reference)
"""Trainium2 Bass kernel for nn_AttnBlockpp3d_old (GroupNorm + 4-head spatial
self-attention + residual), data-parallel over batch across 8 NeuronCores.

Shapes (hardcoded): x [16, 256, 32, 32] f32 -> per core 2 batches of [256, 1024].

Design (v3):
- ScalarE runs ONLY softmax Exp (one act-table load): 64x 1024-wide exps
  ~= 73us; everything else is scheduled around keeping that stream gapless.
- nin biases eliminated by host algebra: b1 cancels in softmax; b2 folds into
  b3' = b3 + W3^T b2; b0 becomes a per-key score bias g(kp) = k0^T b0 computed
  as 4 extra v-projection columns (W1@b0 appended to the W2 stationary) and
  applied via the Exp bias AP.
- GroupNorm rsqrt = quake bit-trick + 1 Newton step on VectorE.
- Softmax denominator rides A@V as a vt ones-column; 1/denom partition
  broadcast via an E-matmul (row-select stationary) + reciprocal after
  broadcast. No DRAM bounce.
- Batch-1 q/k/v projections are woven into batch-0's attention blocks (the
  exp stream leaves PE gaps); low-power K=32 filler matmuls plug remaining
  gaps so the power governor keeps granting full-speed windows.
- PSUM: s0/s1 [128,1024] (2 banks each) rotate scores/qkv/fin/db;
  hh 4x[65,512] accumulators. 8 banks exactly.
"""

import numpy as np

N_CORES = 8
B_TOTAL = 16
B_PER_CORE = B_TOTAL // N_CORES
C = 256
H = 32
S = H * H
NG = 32
NH = 4
CH = C // NH
EPS = 1e-6
SCALE = CH ** -0.5

_CACHE: dict = {}


def _build_nc():
    from contextlib import ExitStack

    import concourse.bacc as bacc
    import concourse.bass as bass
    import concourse.mybir as mybir
    import concourse.tile as tile

    fp32 = mybir.dt.float32
    bf16 = mybir.dt.bfloat16
    i32 = mybir.dt.int32
    AF = mybir.ActivationFunctionType
    OP = mybir.AluOpType
    ts = bass.ts

    nc = bacc.Bacc("TRN2")

    x_d = nc.dram_tensor("x", [B_PER_CORE, C, S], fp32, kind="ExternalInput")
    wq_d = nc.dram_tensor("wq", [128, 2, C], bf16, kind="ExternalInput")
    wk_d = nc.dram_tensor("wk", [128, 2, C], bf16, kind="ExternalInput")
    wv_d = nc.dram_tensor("wv", [128, 2, C + NH], bf16, kind="ExternalInput")
    w3_d = nc.dram_tensor("w3", [128, 2, C], bf16, kind="ExternalInput")
    b3_d = nc.dram_tensor("b3p", [C], fp32, kind="ExternalInput")
    gns_d = nc.dram_tensor("gn_scale", [C], fp32, kind="ExternalInput")
    gnb_d = nc.dram_tensor("gn_bias", [C], fp32, kind="ExternalInput")
    y_d = nc.dram_tensor("y", [B_PER_CORE, C, S], fp32, kind="ExternalOutput")

    with tile.TileContext(nc) as tc, ExitStack() as ctx:
        const = ctx.enter_context(tc.tile_pool(name="const", bufs=1))
        xpool = ctx.enter_context(tc.tile_pool(name="xpool", bufs=1))
        opool = ctx.enter_context(tc.tile_pool(name="opool", bufs=2))
        hpool = ctx.enter_context(tc.tile_pool(name="hpool", bufs=2))
        qkpool = ctx.enter_context(tc.tile_pool(name="qkpool", bufs=1))
        gpool = ctx.enter_context(tc.tile_pool(name="gpool", bufs=16))
        epool = ctx.enter_context(tc.tile_pool(name="epool", bufs=6))
        upool = ctx.enter_context(tc.tile_pool(name="upool", bufs=2))
        npool = ctx.enter_context(tc.tile_pool(name="npool", bufs=4))
        spool = ctx.enter_context(tc.tile_pool(name="spool", bufs=2))
        ps = ctx.enter_context(tc.tile_pool(name="ps", bufs=1, space="PSUM"))

        # ---- phase 0: loads + constants ----
        x_sb = [[None, None], [None, None]]
        for ct in range(2):
            t = xpool.tile([128, S], fp32, tag=f"x0{ct}", name=f"x0{ct}")
            nc.sync.dma_start(out=t, in_=x_d[0, ts(ct, 128), :])
            x_sb[0][ct] = t

        wq = const.tile([128, 2, C], bf16, tag="wq")
        nc.sync.dma_start(out=wq, in_=wq_d[:, :, :])
        wk = const.tile([128, 2, C], bf16, tag="wk")
        nc.sync.dma_start(out=wk, in_=wk_d[:, :, :])
        wv = const.tile([128, 2, C + NH], bf16, tag="wv")
        nc.sync.dma_start(out=wv, in_=wv_d[:, :, :])

        for ct in range(2):
            t = xpool.tile([128, S], fp32, tag=f"x1{ct}", name=f"x1{ct}")
            nc.sync.dma_start(out=t, in_=x_d[1, ts(ct, 128), :])
            x_sb[1][ct] = t

        w3 = const.tile([128, 2, C], bf16, tag="w3")
        nc.sync.dma_start(out=w3, in_=w3_d[:, :, :])

        def col_tiles(dram, name):
            out = []
            for ct in range(2):
                t = const.tile([128, 1], fp32, tag=f"{name}{ct}",
                               name=f"{name}{ct}")
                nc.sync.dma_start(out=t, in_=dram[ts(ct, 128)][:, None])
                out.append(t)
            return out

        gns_sb = col_tiles(gns_d, "gns")
        gnb_sb = col_tiles(gnb_d, "gnb")
        b3_sb = col_tiles(b3_d, "b3")

        # dummy exp: forces the single ACT table load before the stream
        dmy = spool.tile([1, 8], fp32, tag="dmy")
        nc.vector.memset(dmy, 0.0)
        nc.scalar.activation(out=dmy, in_=dmy, func=AF.Exp, scale=1.0)

        # persistent vT tiles with pre-set ones columns (softmax denominator)
        vt_all = [[None] * 8, [None] * 8]
        for b in range(2):
            for j in range(8):
                vt = const.tile([128, NH, CH + 1], bf16, tag=f"vt{b}{j}",
                                name=f"vt{b}{j}")
                nc.gpsimd.memset(vt[:, :, CH:CH + 1], 1.0)
                vt_all[b][j] = vt

        # E [65, 64]: row 64 = 1 else 0 (denominator-broadcast stationary)
        E = const.tile([65, 64], bf16, tag="E")
        nc.gpsimd.memset(E, 0.0)
        nc.gpsimd.memset(E[64:65, :], 1.0)

        # q1[ct] [128, 32]: q1[p, g] = 1 iff group(ct*128+p) == g
        q1 = []
        for ct in range(2):
            t = const.tile([128, NG], fp32, tag=f"q1{ct}", name=f"q1{ct}")
            nc.gpsimd.memset(t, 1.0)
            nc.gpsimd.affine_select(out=t, in_=t, compare_op=OP.is_ge, fill=0.0,
                                    pattern=[[-8, NG]], base=128 * ct,
                                    channel_multiplier=1)
            nc.gpsimd.affine_select(out=t, in_=t, compare_op=OP.is_ge, fill=0.0,
                                    pattern=[[8, NG]], base=7 - 128 * ct,
                                    channel_multiplier=-1)
            q1.append(t)

        # q2[ct] [32, 128]: q2[g, c] = 1 iff group(ct*128+c) == g
        q2 = []
        for ct in range(2):
            t = const.tile([NG, 128], fp32, tag=f"q2{ct}", name=f"q2{ct}")
            nc.gpsimd.memset(t, 1.0)
            base = ct * 128
            nc.gpsimd.affine_select(out=t, in_=t, compare_op=OP.is_ge, fill=0.0,
                                    pattern=[[1, 128]], base=base,
                                    channel_multiplier=-8)
            nc.gpsimd.affine_select(out=t, in_=t, compare_op=OP.is_ge, fill=0.0,
                                    pattern=[[-1, 128]], base=7 - base,
                                    channel_multiplier=8)
            q2.append(t)

        # ---- phase 1 (split): Vector stats chain / qk / vt ----
        h_all = [None, None]
        qk_sb_all = [None, None]
        gsb_all = [[None] * 8, [None] * 8]

        def phase1_pre(b):
            gs_ps = ps.tile([NG, 2], fp32, tag="s0", bufs=1, name="gs_ps")
            for ct in range(2):
                st6 = spool.tile([128, 2, 6], fp32, tag="st6")
                for k in range(2):
                    nc.vector.bn_stats(out=st6[:, k, :],
                                       in_=x_sb[b][ct][:, ts(k, 512)])
                mv = spool.tile([128, 2], fp32, tag="mv")
                nc.vector.bn_aggr(out=mv, in_=st6)
                rhs2 = spool.tile([128, 2], fp32, tag="rhs2")
                nc.vector.tensor_copy(out=rhs2[:, 0:1], in_=mv[:, 0:1])
                nc.vector.tensor_mul(out=rhs2[:, 1:2], in0=mv[:, 0:1],
                                     in1=mv[:, 0:1])
                nc.vector.tensor_add(out=rhs2[:, 1:2], in0=rhs2[:, 1:2],
                                     in1=mv[:, 1:2])
                nc.tensor.matmul(gs_ps, lhsT=q1[ct], rhs=rhs2,
                                 start=(ct == 0), stop=(ct == 1))
            gmv = spool.tile([NG, 2], fp32, tag="gmv")
            nc.vector.tensor_scalar_mul(out=gmv, in0=gs_ps, scalar1=1.0 / 8.0)
            v = spool.tile([NG, 1], fp32, tag="qv")
            nc.vector.tensor_mul(out=v, in0=gmv[:, 0:1], in1=gmv[:, 0:1])
            nc.vector.tensor_tensor(out=v, in0=gmv[:, 1:2], in1=v,
                                    op=OP.subtract)
            nc.vector.tensor_scalar_add(out=v, in0=v, scalar1=EPS)
            # quake rsqrt + 1 Newton step
            t1 = spool.tile([NG, 1], i32, tag="qt1")
            nc.vector.tensor_scalar(out=t1, in0=v.bitcast(i32), scalar1=1,
                                    scalar2=0xFFFFFFFF,
                                    op0=OP.logical_shift_right,
                                    op1=OP.bitwise_xor)
            y0i = spool.tile([NG, 1], i32, tag="qy0")
            nc.vector.tensor_scalar(out=y0i, in0=t1, scalar1=0x5f3759e0,
                                    scalar2=None, op0=OP.add)
            ab_g = spool.tile([NG, 2], fp32, tag="abg")
            y2 = spool.tile([NG, 1], fp32, tag="qy2")
            t3 = spool.tile([NG, 1], fp32, tag="qt3")
            y0 = y0i.bitcast(fp32)
            nc.vector.tensor_mul(out=y2, in0=y0, in1=y0)
            nc.vector.tensor_mul(out=t3, in0=y2, in1=v)
            nc.vector.tensor_scalar(out=t3, in0=t3, scalar1=-0.5,
                                    scalar2=1.5, op0=OP.mult, op1=OP.add)
            nc.vector.tensor_mul(out=ab_g[:, 0:1], in0=y0, in1=t3)
            nc.vector.tensor_mul(out=ab_g[:, 1:2], in0=gmv[:, 0:1],
                                 in1=ab_g[:, 0:1])
            nc.vector.tensor_scalar_mul(out=ab_g[:, 1:2], in0=ab_g[:, 1:2],
                                        scalar1=-1.0)
            h_bf = []
            for ct in range(2):
                ab_ps = ps.tile([128, 2], fp32, tag="s1", bufs=1, name="ab_ps")
                nc.tensor.matmul(ab_ps, lhsT=q2[ct], rhs=ab_g, start=True,
                                 stop=True)
                AB = spool.tile([128, 2], fp32, tag=f"AB{ct}")
                nc.vector.tensor_mul(out=AB[:, 0:1], in0=ab_ps[:, 0:1],
                                     in1=gns_sb[ct])
                nc.vector.tensor_mul(out=AB[:, 1:2], in0=ab_ps[:, 1:2],
                                     in1=gns_sb[ct])
                nc.vector.tensor_add(out=AB[:, 1:2], in0=AB[:, 1:2],
                                     in1=gnb_sb[ct])
                ht = hpool.tile([128, S], bf16, tag=f"h{ct}", name=f"h{b}{ct}")
                nc.vector.tensor_scalar(out=ht, in0=x_sb[b][ct],
                                        scalar1=AB[:, 0:1], scalar2=AB[:, 1:2],
                                        op0=OP.mult, op1=OP.add)
                h_bf.append(ht)
            h_all[b] = h_bf
            qk_sb_all[b] = [[None, None], [None, None]]

        def qk_unit(b, p, dt, evict):
            h_bf = h_all[b]
            wt = wq if p == 0 else wk
            qk_ps = ps.tile([128, S], fp32, tag=f"s{(2 * p + dt) % 2}",
                            bufs=1, name="qk_ps")
            for sc in range(2):
                for ct in range(2):
                    nc.tensor.matmul(
                        qk_ps[:, ts(sc, 512)],
                        lhsT=wt[:, ct, ts(dt, 128)],
                        rhs=h_bf[ct][:, ts(sc, 512)],
                        start=(ct == 0), stop=(ct == 1))
            t = qkpool.tile([128, S], bf16, tag=f"qk{b}{p}{dt}",
                            name=f"qk{b}{p}{dt}")
            if evict == "scalar":
                nc.scalar.copy(out=t, in_=qk_ps)
            else:
                nc.vector.tensor_copy(out=t, in_=qk_ps)
            qk_sb_all[b][p][dt] = t

        def vt_unit(b, j, evict):
            h_bf = h_all[b]
            vt_ps = ps.tile([128, C + NH], fp32, tag=f"s{j % 2}",
                            bufs=1, name="vt_ps")
            for ct in range(2):
                nc.tensor.matmul(vt_ps, lhsT=h_bf[ct][:, ts(j, 128)],
                                 rhs=wv[:, ct, :], start=(ct == 0),
                                 stop=(ct == 1))
            vt = vt_all[b][j]
            if evict == "scalar":
                nc.scalar.copy(
                    out=vt[:, :, 0:CH],
                    in_=vt_ps[:, 0:C].rearrange("p (h c) -> p h c", h=NH))
            else:
                nc.vector.tensor_copy(
                    out=vt[:, :, 0:CH],
                    in_=vt_ps[:, 0:C].rearrange("p (h c) -> p h c", h=NH))
            gsb = gpool.tile([128, NH], fp32, tag="gsb")
            nc.vector.tensor_scalar_mul(out=gsb, in0=vt_ps[:, C:C + NH],
                                        scalar1=SCALE)
            gsb_all[b][j] = gsb

        # ---- phase 2 helpers ----
        hh_n_all = {}

        def scores_exp(b, pr, j, hp, s_tiles):
            qk_sb = qk_sb_all[b]
            s_ps = ps.tile([128, S], fp32, tag=f"s{hp}", bufs=1, name="s_ps")
            for sc in range(2):
                nc.tensor.matmul(
                    s_ps[:, ts(sc, 512)],
                    lhsT=qk_sb[1][pr][ts(hp, CH), ts(j, 128)],
                    rhs=qk_sb[0][pr][ts(hp, CH), ts(sc, 512)],
                    start=True, stop=True)
            et = epool.tile([128, S], bf16, tag="e", name=f"et{j}{hp}")
            nc.scalar.activation(out=et, in_=s_ps, func=AF.Exp,
                                 bias=gsb_all[b][j][:, 2 * pr + hp:2 * pr + hp + 1],
                                 scale=SCALE)
            s_tiles[(j, hp)] = et

        def av(b, pr, j, hp, s_tiles, hh_ps):
            vt = vt_all[b][j]
            et = s_tiles[(j, hp)]
            for sc in range(2):
                nc.tensor.matmul(hh_ps[hp][sc],
                                 lhsT=vt[:, 2 * pr + hp, :],
                                 rhs=et[:, ts(sc, 512)],
                                 start=(j == 0), stop=(j == 7))

        def normalize(b, pr, hp, hh_ps):
            hh_b = upool.tile([CH + 1, S], bf16, tag="hhb", name="hh_b")
            rd = upool.tile([CH, S], fp32, tag="rd", name="rd")
            hh_n = hh_n_all[(b, pr)]
            for sc in range(2):
                nc.vector.tensor_copy(out=hh_b[:, ts(sc, 512)],
                                      in_=hh_ps[hp][sc])
            db_ps = [ps.tile([CH, 512], fp32, tag=f"s{sc}", bufs=1,
                             name=f"db{sc}") for sc in range(2)]
            for sc in range(2):
                nc.tensor.matmul(db_ps[sc], lhsT=E,
                                 rhs=hh_b[:, ts(sc, 512)], start=True,
                                 stop=True)
            for sc in range(2):
                nc.vector.reciprocal_approx_fast(out=rd[:, ts(sc, 512)],
                                                 in_=db_ps[sc])
            for sc in range(2):
                nc.vector.tensor_mul(out=hh_n[ts(hp, CH), ts(sc, 512)],
                                     in0=hh_ps[hp][sc][0:CH, :],
                                     in1=rd[:, ts(sc, 512)])

        def fin(b, dt):
            fin_ps = ps.tile([128, S], fp32, tag=f"s{dt}", bufs=1,
                             name="fin_ps")
            for sc in range(2):
                for ct in range(2):
                    nc.tensor.matmul(
                        fin_ps[:, ts(sc, 512)],
                        lhsT=w3[:, ct, ts(dt, 128)],
                        rhs=hh_n_all[(b, ct)][:, ts(sc, 512)],
                        start=(ct == 0), stop=(ct == 1))
            out_t = opool.tile([128, S], fp32, tag=f"out{dt}", name=f"o{dt}")
            nc.vector.scalar_tensor_tensor(out=out_t, in0=fin_ps,
                                           scalar=b3_sb[dt], in1=x_sb[b][dt],
                                           op0=OP.add, op1=OP.add)
            nc.sync.dma_start(out=y_d[b, ts(dt, 128), :], in_=out_t)

        # ---- emission schedule ----
        phase1_pre(0)
        for p in range(2):
            for dt in range(2):
                qk_unit(0, p, dt, "scalar")
        for j in range(8):
            vt_unit(0, j, "scalar")
        phase1_pre(1)

        blocks = [(0, 0), (0, 1), (1, 0), (1, 1)]
        # extra woven work per block index: {j: [callback, ...]}
        extras = {
            0: {2: [lambda: qk_unit(1, 0, 0, "vector")],
                3: [lambda: qk_unit(1, 0, 1, "vector")],
                4: [lambda: qk_unit(1, 1, 0, "vector")],
                5: [lambda: qk_unit(1, 1, 1, "vector")],
                6: [lambda: vt_unit(1, 0, "vector")],
                7: [lambda: vt_unit(1, 1, "vector")]},
            1: {j: [lambda j=j: vt_unit(1, j + 2, "vector")]
                for j in range(6)},
            2: {},
            3: {},
        }

        prev = None
        prev_state = None
        for bi, (b, pr) in enumerate(blocks):
            hh_n_all[(b, pr)] = npool.tile([128, S], bf16, tag="hhn",
                                           name=f"hhn{b}{pr}")
            s_tiles = {}
            hh_ps = None
            ext = extras[bi]
            for j in range(8):
                if prev is not None:
                    pb, ppr = prev
                    if j == 0:
                        for hp in range(2):
                            av(pb, ppr, 7, hp, prev_state[0], prev_state[1])
                    elif j == 1:
                        for hp in range(2):
                            normalize(pb, ppr, hp, prev_state[1])
                    elif j == 2 and ppr == 1:
                        fin(pb, 0)
                    elif j == 3 and ppr == 1:
                        fin(pb, 1)
                for hp in range(2):
                    scores_exp(b, pr, j, hp, s_tiles)
                for fn in ext.get(j, []):
                    fn()
                if j >= 1 and j not in ext:
                    qk0 = qk_sb_all[b][0][pr]
                    for hp in range(2):
                        f_ps = ps.tile([32, 512], fp32, tag=f"s{hp}",
                                       bufs=1, name="f_ps")
                        nc.tensor.matmul(f_ps, lhsT=qk0[0:32, 0:32],
                                         rhs=qk0[0:32, 0:512],
                                         start=True, stop=True)
                if j == 1:
                    hh_ps = [[ps.tile([CH + 1, 512], fp32, tag="hh", bufs=4,
                                      name=f"hh{hp}{sc}") for sc in range(2)]
                             for hp in range(2)]
                if j >= 1:
                    for hp in range(2):
                        av(b, pr, j - 1, hp, s_tiles, hh_ps)
            prev = (b, pr)
            prev_state = (s_tiles, hh_ps)

        pb, ppr = prev
        for hp in range(2):
            av(pb, ppr, 7, hp, prev_state[0], prev_state[1])
        for hp in range(2):
            normalize(pb, ppr, hp, prev_state[1])
        fin(pb, 0)
        fin(pb, 1)

    nc.finalize()
    return nc


def _pack_weights(inputs):
    """Host-side algebra + bf16 packing. Returns dict of shared arrays."""
    import ml_dtypes

    W0 = np.asarray(inputs["W0"], np.float32)
    b0 = np.asarray(inputs["b0"], np.float32)
    W1 = np.asarray(inputs["W1"], np.float32)
    W2 = np.asarray(inputs["W2"], np.float32)
    b2 = np.asarray(inputs["b2"], np.float32)
    W3 = np.asarray(inputs["W3"], np.float32)
    b3 = np.asarray(inputs["b3"], np.float32)

    def pack(w):
        return np.ascontiguousarray(
            w.reshape(2, 128, -1).transpose(1, 0, 2)).astype(ml_dtypes.bfloat16)

    G = np.zeros((C, NH), np.float32)
    for h in range(NH):
        G[:, h] = W1[:, h * CH:(h + 1) * CH] @ b0[h * CH:(h + 1) * CH]
    wv_ext = np.concatenate([W2, G], axis=1)
    b3p = b3 + W3.T @ b2

    return {
        "wq": pack(W0),
        "wk": pack(W1),
        "wv": pack(wv_ext),
        "w3": pack(W3),
        "b3p": np.ascontiguousarray(b3p, np.float32),
        "gn_scale": np.ascontiguousarray(np.asarray(inputs["gn_scale"], np.float32)),
        "gn_bias": np.ascontiguousarray(np.asarray(inputs["gn_bias"], np.float32)),
    }


def _in_maps(inputs):
    x = np.ascontiguousarray(np.asarray(inputs["x"], dtype=np.float32))
    B = x.shape[0]
    xr = x.reshape(B, C, S)
    shared = _pack_weights(inputs)
    maps = []
    for core in range(N_CORES):
        m = dict(shared)
        m["x"] = np.ascontiguousarray(xr[core * B_PER_CORE:(core + 1) * B_PER_CORE])
        maps.append(m)
    return maps


def kernel(**inputs: np.ndarray) -> np.ndarray:
    from concourse.bass_utils import run_bass_kernel_spmd

    if "nc" not in _CACHE:
        _CACHE["nc"] = _build_nc()
    res = run_bass_kernel_spmd(_CACHE["nc"], _in_maps(inputs),
                               core_ids=list(range(N_CORES)))
    out = np.concatenate([res.results[c]["y"] for c in range(N_CORES)], axis=0)
    B = np.asarray(inputs["x"]).shape[0]
    return out.reshape(B, C, H, H).astype(np.float32)


def run_profiled(inputs):
    """Like kernel() but with trace=True; returns (out, exec_time_ns)."""
    from concourse.bass_utils import run_bass_kernel_spmd

    if "nc" not in _CACHE:
        _CACHE["nc"] = _build_nc()
    res = run_bass_kernel_spmd(_CACHE["nc"], _in_maps(inputs),
                               core_ids=list(range(N_CORES)), trace=True)
    out = np.concatenate([res.results[c]["y"] for c in range(N_CORES)], axis=0)
    B = np.asarray(inputs["x"]).shape[0]
    return out.reshape(B, C, H, H).astype(np.float32), res.exec_time_ns
